# revision 52
# baseline (speedup 1.0000x reference)
"""EquivariantAttention Trainium2 kernel.

Reference computation (B=4, S=512, D=512, H=8, DH=64):
    qkv = x @ W_qkv                      -> q, k, v  (b, s, h, dh)
    geo_w = geometric_features @ W_geo   -> (b, h, i, j)
    pos_w = positional_encodings @ W_pos -> (h, i, j)
    scores = q k^T / sqrt(dh) + geo_w + pos_w
    attn   = softmax_j(scores)            (mask is all-ones -> no-op)
    out    = (attn @ v) @ W_out

Sharding: the 512 MB positional_encodings table dominates HBM traffic, so the
query dim i is sharded across the 8 cores (64 rows each).  Every core computes
its own i-slice of the output; the host concats.

Design notes (per core):
- Scores live TRANSPOSED — j on partitions, (h, i) in the free dim — because
  pos_w can only leave the tensor engine as out[M=j_tile, N=h] with d on the
  contraction partitions.  The host stages positional_encodings pre-transposed
  to (d, i, j) and pre-cast to fp8 e3m4 (4 mantissa bits keep the pos_w
  quantization error ~0.9%), quartering the dominant HBM stream vs f32; the
  fp8 stationary x bf16 W_pos mix is exact on the PE.
- The k projection over all 2048 tokens never happens: scores are computed as
  x^T (W_k q) — Z = W_k q is an 8-head batch of tiny matmuls, and the j-side
  contraction reuses the xT tiles already resident for the q/v projections.
- geo_w is combined with pos_w into per-(b, j-block) `posgeo` bias tiles
  (wide batched DVE/Pool ops), so the softmax tail is just add + exp per
  (b, jb) — no separate geo multiply.  pos_w accumulates in two i-halves so
  half the bias work runs under the DMA stream.
- v tiles carry a ones column per head: attn @ v and the softmax denominator
  come out of one matmul pass.  Softmax over j (= partitions) skips
  max-subtraction (scores are O(1)).  The out-projection runs per batch,
  overlapping the next batch's attention.
- The whole body sits in a hardware For_i loop (NREPS iterations per NEFF
  execution, all-engine barrier + sem reset between iterations).  Each
  iteration is a complete independent kernel run — the loop only amortizes
  the multi-ms axon per-call dispatch overhead so test.py can measure
  per-iteration HW latency precisely.
"""

import numpy as np

B, S, D, H = 4, 512, 512, 8
DH = D // H            # 64
NCORES = 8
IS = S // NCORES       # 64  i-rows per core
T = B * S              # 2048 tokens
TI = B * IS            # 256 slice tokens
IGRP = 8               # i-rows per P-load DMA group
# In-NEFF repetition count: the body below is emitted once and wrapped in a
# hardware For_i loop, so one NEFF execution runs the full kernel NREPS times
# back-to-back (all-engine barrier + sem reset between iterations — no
# overlap).  Each iteration is a complete, self-contained kernel run; the
# repetition only amortizes the multi-ms per-call dispatch overhead of the
# axon-tunnelled PJRT path, letting test.py measure per-iteration HW latency.
NREPS = 2048

_CACHE = {}


def _build_program(nreps: int = NREPS):
    from contextlib import ExitStack

    import concourse.bacc as bacc
    import concourse.mybir as mybir
    import concourse.tile as tile
    from concourse.masks import make_identity

    f32 = mybir.dt.float32
    bf16 = mybir.dt.bfloat16
    AF = mybir.ActivationFunctionType
    ALU = mybir.AluOpType

    nc = bacc.Bacc(
        "TRN2",
        target_bir_lowering=False,
        debug=False,
        enable_asserts=False,
        num_devices=NCORES,
    )

    x_d = nc.dram_tensor("x", [D, T], bf16, kind="ExternalInput").ap()
    xs_d = nc.dram_tensor("x_slice", [D, TI], bf16, kind="ExternalInput").ap()
    # W_k^T, head-major: row dh, col h*512 + d  (the k-block of W_qkv,
    # host-transposed).  Scores are computed as x^T (W_k q) instead of
    # (W_k^T x)·q — the k projection over all 2048 tokens never happens.
    wkT_d = nc.dram_tensor("w_kT", [DH, H * D], bf16, kind="ExternalInput").ap()
    f8 = mybir.dt.float8e3
    # positional_encodings arrive host-pre-transposed to (d, i, j) so the
    # contraction dim d lands on SBUF partitions straight out of the DMA;
    # fp8 e3m4 halves the dominant HBM stream (4 mantissa bits keep the
    # pos_w quantization error ~0.9%; pos values ~N(0,1) fit the e3m4
    # range, and W_pos is pre-scaled by POS_SCALE to reach normals)
    p_d = nc.dram_tensor("pos_enc", [D, IS, S], f8, kind="ExternalInput").ap()
    g_d = nc.dram_tensor("geo", [B, 2, IS, S], bf16, kind="ExternalInput").ap()
    # q and v blocks only (k is folded into wkT): cols 0:512 = W_q, 512:1024 = W_v
    wqkv_d = nc.dram_tensor("w_qkv", [D, 2 * D], bf16, kind="ExternalInput").ap()
    wpos_d = nc.dram_tensor("w_pos", [D, H], bf16, kind="ExternalInput").ap()
    wgeo_d = nc.dram_tensor("w_geo", [2, H], f32, kind="ExternalInput").ap()
    wout_d = nc.dram_tensor("w_out", [D, D], bf16, kind="ExternalInput").ap()
    out_d = nc.dram_tensor("out", [B, IS, D], f32, kind="ExternalOutput").ap()

    with tile.TileContext(nc) as tc, ExitStack() as ctx:
        if nreps > 1:
            ctx.enter_context(tc.For_i(0, nreps))
        # ---------------- Phase 0: constants ----------------
        cp = ctx.enter_context(tc.tile_pool(name="consts", bufs=1))

        ident = cp.tile([128, 128], bf16, name="ident", tag="ident")
        make_identity(nc, ident)

        ones_col = cp.tile([128, 1], bf16, name="ones_col", tag="ones_col")
        nc.gpsimd.memset(ones_col, 1.0)

        wqkv_sb = []
        for dt_ in range(4):
            t_ = cp.tile([128, 2 * D], bf16, name=f"wqkv_{dt_}", tag=f"wqkv{dt_}")
            nc.scalar.dma_start(out=t_, in_=wqkv_d[dt_ * 128:(dt_ + 1) * 128, :])
            wqkv_sb.append(t_)

        wkT_sb = cp.tile([DH, H * D], bf16, name="wkT_sb", tag="wkT")
        nc.scalar.dma_start(out=wkT_sb, in_=wkT_d)

        # W_pos as (128, 4*8): [:, db*8:(db+1)*8] = rows db*128..db*128+127
        # (bf16 moving operand vs fp8 stationary table: PE allows the mix,
        # and keeping the weight exact halves the pos_w quantization error)
        wpos_sb = cp.tile([128, 32], bf16, name="wpos_sb", tag="wpos")
        nc.scalar.dma_start(
            out=wpos_sb.rearrange("p (a h) -> p a h", a=4),
            in_=wpos_d.rearrange("(a p) h -> p a h", p=128),
        )

        # W_geo broadcast to all partitions: (128, 16) f32, col c*8+h
        wgeo_flat = cp.tile([1, 16], f32, name="wgeo_flat", tag="wgf")
        nc.scalar.dma_start(
            out=wgeo_flat, in_=wgeo_d.rearrange("c h -> (c h)")[None, :])
        wgeo_fbf = cp.tile([1, 16], bf16, name="wgeo_fbf", tag="wgfb")
        nc.vector.tensor_copy(wgeo_fbf, wgeo_flat)
        wgeo_bc = cp.tile([128, 16], f32, name="wgeo_bc", tag="wgbc")
        ones_r128 = cp.tile([1, 128], bf16, name="ones_r128", tag="ones_r128")
        nc.gpsimd.memset(ones_r128, 1.0)
        with tc.tile_pool(name="bc_ps2", bufs=1, space="PSUM") as bc_pool:
            bc_ps = bc_pool.tile([128, 16], f32, name="bc_ps2", tag="bcps2")
            nc.tensor.matmul(bc_ps, ones_r128, wgeo_fbf, start=True, stop=True)
            nc.vector.tensor_copy(wgeo_bc, bc_ps)

        wout_sb = []
        for db in range(4):
            t_ = cp.tile([128, D], bf16, name=f"wout_{db}", tag=f"wout{db}")
            nc.scalar.dma_start(out=t_, in_=wout_d[db * 128:(db + 1) * 128, :])
            wout_sb.append(t_)

        # ---------------- Phase 1: xT, xsT straight from host ------------
        # x arrives host-transposed (d, tokens) bf16: plain DMAs, no PE work
        xT_sb = [cp.tile([128, T], bf16, name=f"xT_{db}", tag=f"xT{db}")
                 for db in range(4)]
        xsT_sb = [cp.tile([128, TI], bf16, name=f"xsT_{db}", tag=f"xsT{db}")
                  for db in range(4)]
        for db in range(4):
            nc.scalar.dma_start(out=xT_sb[db], in_=x_d[db * 128:(db + 1) * 128, :])
            nc.scalar.dma_start(out=xsT_sb[db], in_=xs_d[db * 128:(db + 1) * 128, :])

        # ---------------- Phase 2 decls (work emitted inside phase 4) ----
        # v tiles carry 65 cols per head: 64 v values + a ones column, so the
        # attn matmul computes the softmax denominator in the same pass
        v_sb = [cp.tile([128, H * 65], bf16, name=f"v_{tt}", tag=f"v{tt}")
                for tt in range(T // 128)]
        qT_sb = [cp.tile([DH, TI], bf16, name=f"qT_{h}", tag=f"qT{h}")
                 for h in range(H)]
        # Z[db] : (128 d-chunk, (b,h,i) 2048) bf16 = W_k q (scores stage 1)
        z_sb = [cp.tile([128, B * 512], bf16, name=f"z_{db}", tag=f"z{db}")
                for db in range(4)]
        # qk_sb[b][jb] : (128 j, (h,i) 512) bf16 raw q·k/8 scores
        qk_sb = [[cp.tile([128, 512], bf16, name=f"qk{b}_{jb}",
                          tag=f"qk{b}{jb}") for jb in range(4)]
                 for b in range(B)]

        def _emit_v(proj_pool, tt):
            ps = proj_pool.tile([128, 512], f32, name="ps_v", tag="pj")
            for dt_ in range(4):
                nc.tensor.matmul(
                    ps,
                    xT_sb[dt_][:, tt * 128:(tt + 1) * 128],
                    wqkv_sb[dt_][:, 512:1024],
                    start=(dt_ == 0), stop=(dt_ == 3),
                )
            vv = v_sb[tt].rearrange("p (h c) -> p h c", c=65)
            pv = ps.rearrange("p (h d) -> p h d", d=DH)
            if tt % 2 == 0:
                nc.vector.tensor_copy(vv[:, :, 0:DH], pv)
            else:
                nc.scalar.copy(vv[:, :, 0:DH], pv)
            nc.gpsimd.memset(vv[:, :, DH:65], 1.0)

        def _emit_qT(proj_pool, h):
            ps = proj_pool.tile([DH, TI], f32, name="ps_q", tag="pj")
            for dt_ in range(4):
                nc.tensor.matmul(
                    ps,
                    wqkv_sb[dt_][:, h * DH:(h + 1) * DH],
                    xsT_sb[dt_],
                    start=(dt_ == 0), stop=(dt_ == 3),
                )
            nc.scalar.mul(qT_sb[h], ps, 0.125)   # fold 1/sqrt(DH)

        def _emit_z(proj_pool, h, db):
            ps = proj_pool.tile([128, TI], f32, name="ps_z", tag="pj")
            nc.tensor.matmul(
                ps,
                wkT_sb[:, h * D + db * 128:h * D + (db + 1) * 128],
                qT_sb[h],
                start=True, stop=True,
            )
            zv = z_sb[db].rearrange(
                "p (b h i) -> p b h i", h=H, i=IS)[:, :, h, :]
            pv = ps.rearrange("p (b i) -> p b i", i=IS)
            if (h + db) % 2 == 0:
                nc.vector.tensor_copy(zv, pv)
            else:
                nc.scalar.copy(zv, pv)

        def _emit_s(proj_pool, b, jb):
            ps = proj_pool.tile([128, 512], f32, name="ps_s", tag="pj")
            for db in range(4):
                nc.tensor.matmul(
                    ps,
                    xT_sb[db][:, b * S + jb * 128:b * S + (jb + 1) * 128],
                    z_sb[db][:, b * 512:(b + 1) * 512],
                    start=(db == 0), stop=(db == 3),
                )
            if (b + jb) % 2 == 0:
                nc.vector.tensor_copy(qk_sb[b][jb], ps)
            else:
                nc.scalar.copy(qk_sb[b][jb], ps)

        # geo: the two channel planes are PE-transposed into two WIDE tiles
        # (all 16 (b,jb) units side by side), the per-head combine then runs
        # as 16 full-width DVE ops instead of 256 64-col ones, and the result
        # is pre-ADDED to pos_w (posgeo tiles) so the softmax tail needs no
        # separate geo multiply and no geo exp.
        gT_all = [cp.tile([128, 1024], bf16, name=f"gT_all{c}", tag=f"gA{c}")
                  for c in range(2)]
        t2_all = cp.tile([128, 1024], f32, name="t2_all", tag="t2A")
        gtmp_all = cp.tile([128, 8192], bf16, name="gtmp_all", tag="gtA")
        posgeo = [[cp.tile([128, 512], bf16, name=f"pg{b}_{jb}",
                           tag=f"pg{b}{jb}") for jb in range(4)]
                  for b in range(B)]
        g_nat = []
        for b in range(B):
            gn2 = []
            for c in range(2):
                gn = cp.tile([IS, S], bf16, name=f"g_nat{b}_{c}", tag=f"gn{b}{c}")
                nc.scalar.dma_start(out=gn, in_=g_d[b, c])
                gn2.append(gn)
            g_nat.append(gn2)

        def _emit_geoT(proj_pool, b, jb):
            u = b * 4 + jb
            for c in range(2):
                # PE transpose (not xbar DMA-transpose: mixing DmaTranspose
                # with the P-load DMA stream forces xbar-mode serialization);
                # psum comes from the shared proj pool's slots
                gt_ps = proj_pool.tile([128, IS], bf16,
                                       name=f"gt_ps{c}", tag="pj")
                nc.tensor.transpose(
                    gt_ps, g_nat[b][c][:, jb * 128:(jb + 1) * 128],
                    ident[0:IS, 0:IS])
                dst = gT_all[c][:, u * IS:(u + 1) * IS]
                if c == 0:
                    nc.vector.tensor_copy(dst, gt_ps)
                else:
                    nc.scalar.copy(dst, gt_ps)

        def _emit_gtmp(h):
            gv = gtmp_all.rearrange("p (u h i) -> p u h i", h=H, i=IS)
            g0 = gT_all[0].rearrange("p (u i) -> p u i", i=IS)
            g1 = gT_all[1].rearrange("p (u i) -> p u i", i=IS)
            t2 = t2_all.rearrange("p (u i) -> p u i", i=IS)
            nc.vector.tensor_scalar(
                t2, g1, wgeo_bc[:, 8 + h:9 + h], None, op0=ALU.mult)
            nc.vector.scalar_tensor_tensor(
                gv[:, :, h, :], g0, wgeo_bc[:, h:h + 1], t2,
                op0=ALU.mult, op1=ALU.add)

        pos_sb = [cp.tile([128, 512], bf16, name=f"pos_{jb}", tag=f"pos{jb}")
                  for jb in range(4)]

        def _emit_poscopy(half, jb):
            # single strided (i,h) -> (h,i) re-layout pass per (jb, half);
            # everything downstream then reads contiguous blocks
            hi0 = half * (IS // 2)
            hi1 = hi0 + IS // 2
            pp = pos_ps[jb].rearrange("p (i h) -> p h i", h=H)[:, :, hi0:hi1]
            og = pos_sb[jb].rearrange("p (h i) -> p h i", i=IS)[:, :, hi0:hi1]
            nc.vector.tensor_copy(og, pp)

        def _emit_posgeo(half, b, jb):
            # b-dependent geo bias add; mid-stream half rides the idle Pool
            # engine, the latency-critical tail half alternates Pool/DVE
            hi0 = half * (IS // 2)
            hi1 = hi0 + IS // 2
            u = b * 4 + jb
            pp = pos_sb[jb].rearrange("p (h i) -> p h i", i=IS)[:, :, hi0:hi1]
            gg = gtmp_all.rearrange(
                "p (u h i) -> p u h i", h=H, i=IS)[:, u, :, hi0:hi1]
            og = posgeo[b][jb].rearrange(
                "p (h i) -> p h i", i=IS)[:, :, hi0:hi1]
            eng = nc.gpsimd if (half == 0 or (b + jb) % 2 == 0) else nc.vector
            eng.tensor_add(og, pp, gg)

        proj_items = (
            [lambda pp, tt=tt: _emit_v(pp, tt) for tt in range(T // 128)]
            + [it
               for h in range(H)
               for it in ([lambda pp, h=h: _emit_qT(pp, h)]
                          + [lambda pp, h=h, db=db: _emit_z(pp, h, db)
                             for db in range(4)])]
            + [lambda pp, b=b, jb=jb: _emit_s(pp, b, jb)
               for b in range(B) for jb in range(4)]
        )
        geoT_items = [(b, jb) for b in range(B) for jb in range(4)]

        # ---------------- Phase 4: pos_w + interleaved projections -------
        # The PE stream is in-order: interleaving projection matmuls between
        # each i-group's (DMA-gated) pos matmuls lets the PE fill DMA-wait
        # gaps with useful work instead of stalling.  pos_ps accumulates in
        # two i-halves so the posgeo adds for i<32 run mid-stream.
        with tc.tile_pool(name="pos_ps", bufs=1, space="PSUM") as pos_pool:
            pos_ps = [pos_pool.tile([128, 512], f32, name=f"pos_ps{jb}",
                                    tag=f"pps{jb}") for jb in range(4)]
            with tc.tile_pool(name="p_t", bufs=12) as pt_pool, \
                 tc.tile_pool(name="proj_ps", bufs=4, space="PSUM") as proj_pool:
                n_groups = IS // IGRP
                per_grp = (len(proj_items) + n_groups - 1) // n_groups
                HALF = IS // 2
                for gi, i0 in enumerate(range(0, IS, IGRP)):
                    ptg = []
                    for db in range(4):
                        pt = pt_pool.tile([128, IGRP * 512], f8,
                                          name="ptg", tag="ptg")
                        nc.sync.dma_start(
                            out=pt.rearrange("p (a j) -> p a j", a=IGRP),
                            in_=p_d[db * 128:(db + 1) * 128, i0:i0 + IGRP, :],
                        )
                        ptg.append(pt)
                    for it in proj_items[gi * per_grp:(gi + 1) * per_grp]:
                        it(proj_pool)
                    if gi < 3:
                        for b, jb in geoT_items[gi * 6:(gi + 1) * 6]:
                            _emit_geoT(proj_pool, b, jb)
                    elif gi == 3:
                        for h in range(H):
                            _emit_gtmp(h)
                    elif gi == 4:
                        for jb in range(4):
                            _emit_poscopy(0, jb)
                    elif gi == 5:
                        for b in range(B):
                            for jb in range(4):
                                _emit_posgeo(0, b, jb)
                    for a in range(IGRP):
                        i = i0 + a
                        for jb in range(4):
                            for db in range(4):
                                nc.tensor.matmul(
                                    pos_ps[jb][:, i * 8:(i + 1) * 8],
                                    ptg[db][:, a * 512 + jb * 128:
                                            a * 512 + (jb + 1) * 128],
                                    wpos_sb[:, db * 8:(db + 1) * 8],
                                    # one psum group per (bank, i-half): the
                                    # half's first i starts, its last i
                                    # stops; each i's first db write lands on
                                    # pending-zero bytes and overwrites,
                                    # later dbs accumulate.
                                    start=(i % HALF == 0 and db == 0),
                                    stop=(i % HALF == HALF - 1 and db == 3),
                                )
            for jb in range(4):
                _emit_poscopy(1, jb)
            for b in range(B):
                for jb in range(4):
                    _emit_posgeo(1, b, jb)

        # ---------------- Phase 5: scores, softmax, attn @ v -------------
        O_sb = [cp.tile([IS, D], bf16, name=f"O_{b}", tag=f"O{b}") for b in range(B)]
        with tc.tile_pool(name="o_ps", bufs=1, space="PSUM") as o_pool, \
             tc.tile_pool(name="ot_ps", bufs=2, space="PSUM") as ot_pool, \
             tc.tile_pool(name="f_ps", bufs=2, space="PSUM") as f_pool, \
             tc.tile_pool(name="att_sb", bufs=3) as att_pool:
            for b in range(B):
                o_ps2 = [o_pool.tile([IS, 4 * 65], f32, name=f"o_ps{g}",
                                     tag=f"ops{g}", bufs=2) for g in range(2)]
                for jb in range(4):
                    t1 = att_pool.tile([128, 512], f32, name="t1", tag="t1")
                    nc.vector.tensor_add(t1, qk_sb[b][jb], posgeo[b][jb])
                    e1 = att_pool.tile([128, 512], bf16, name="e1", tag="e1")
                    nc.scalar.activation(e1, t1, AF.Exp)
                    tt = b * 4 + jb
                    for h in range(H):
                        g, hg = divmod(h, 4)
                        nc.tensor.matmul(
                            o_ps2[g][:, hg * 65:(hg + 1) * 65],
                            e1[:, h * IS:(h + 1) * IS],
                            v_sb[tt][:, h * 65:(h + 1) * 65],
                            start=(jb == 0 and hg == 0),
                            stop=(jb == 3 and hg == 3),
                        )
                recip = att_pool.tile([IS, H], f32, name="recip", tag="recip")
                for g in range(2):
                    nc.vector.reciprocal(
                        recip[:, g * 4:(g + 1) * 4],
                        o_ps2[g].rearrange("p (h c) -> p h c", c=65)[:, :, DH])
                for h in range(H):
                    g, hg = divmod(h, 4)
                    src = o_ps2[g][:, hg * 65:hg * 65 + DH]
                    if h % 2 == 0:
                        nc.vector.tensor_scalar(
                            O_sb[b][:, h * DH:(h + 1) * DH], src,
                            recip[:, h:h + 1], None, op0=ALU.mult)
                    else:
                        nc.scalar.activation(
                            O_sb[b][:, h * DH:(h + 1) * DH], src,
                            AF.Copy, scale=recip[:, h:h + 1])
                # out-proj for this b right away: overlaps later batches'
                # attention instead of forming a serial phase at the end
                otb = []
                for db in range(4):
                    ot_ps_t = ot_pool.tile([128, IS], bf16, name="otps",
                                           tag="otps")
                    nc.tensor.transpose(
                        ot_ps_t, O_sb[b][:, db * 128:(db + 1) * 128],
                        ident[0:IS, 0:IS])
                    ot = att_pool.tile([128, IS], bf16, name=f"ot{db}",
                                       tag=f"ot{db}", bufs=2)
                    if db % 2 == 0:
                        nc.vector.tensor_copy(ot, ot_ps_t)
                    else:
                        nc.scalar.copy(ot, ot_ps_t)
                    otb.append(ot)
                f_ps = f_pool.tile([IS, D], f32, name="f_ps", tag="fps")
                for db in range(4):
                    nc.tensor.matmul(
                        f_ps, otb[db], wout_sb[db],
                        start=(db == 0), stop=(db == 3),
                    )
                fout = att_pool.tile([IS, D], f32, name="fout", tag="fout",
                                     bufs=2)
                nc.vector.tensor_copy(fout, f_ps)
                nc.sync.dma_start(out=out_d[b], in_=fout)

    nc.compile()
    return nc


def _get_program(nreps: int = NREPS):
    key = f"nc{nreps}"
    if key not in _CACHE:
        _CACHE[key] = _build_program(nreps)
    return _CACHE[key]


def make_in_maps(inputs):
    import ml_dtypes
    bf = ml_dtypes.bfloat16
    f8 = ml_dtypes.float8_e3m4
    x = np.asarray(inputs["x"], np.float32)                       # (B, S, D)
    geo = np.asarray(inputs["geometric_features"], np.float32)    # (B, S, S, 2)
    pos = np.asarray(inputs["positional_encodings"], np.float32)  # (S, S, D)
    wqkv = np.asarray(inputs["W_qkv"], np.float32)
    wout = np.asarray(inputs["W_out"], np.float32)
    wgeo = np.asarray(inputs["W_geo"], np.float32)
    wpos = np.asarray(inputs["W_pos"], np.float32)
    mask = np.asarray(inputs["mask"])

    assert mask.all(), "kernel assumes an all-true mask"
    for k in ("b_qkv", "b_out", "b_geo", "b_pos"):
        assert not np.asarray(inputs[k], np.float32).any(), \
            "kernel assumes zero biases (reference setup_inputs uses zeros)"

    # big inputs staged as bf16 on the host: halves device HBM traffic and
    # makes every load a plain HWDGE DMA (matmuls consume bf16 anyway, so
    # numerics match the previous cast-during-DMA scheme)
    x_flat = np.ascontiguousarray(x.reshape(T, D).T.astype(bf))
    # q and v blocks side by side; the k block ships transposed (head-major)
    wqkv_b = np.ascontiguousarray(
        np.concatenate([wqkv[:, :D], wqkv[:, 2 * D:]], axis=1).astype(bf))
    wkT_b = np.ascontiguousarray(
        wqkv[:, D:2 * D].reshape(D, H, DH).transpose(2, 1, 0)
        .reshape(DH, H * D).astype(bf))
    wpos_b = np.ascontiguousarray(wpos.astype(bf))
    wout_b = np.ascontiguousarray(wout.astype(bf))
    in_maps = []
    for c in range(NCORES):
        lo = c * IS
        in_maps.append({
            "x": x_flat,
            "x_slice": np.ascontiguousarray(
                x[:, lo:lo + IS].reshape(TI, D).T.astype(bf)),
            "pos_enc": np.ascontiguousarray(
                pos[lo:lo + IS].transpose(2, 0, 1).astype(f8)),
            "geo": np.ascontiguousarray(
                geo[:, lo:lo + IS].transpose(0, 3, 1, 2).astype(bf)),
            "w_qkv": wqkv_b,
            "w_kT": wkT_b,
            "w_pos": wpos_b,
            "w_geo": wgeo,
            "w_out": wout_b,
        })
    return in_maps


def gather_out(results):
    out = np.empty((B, S, D), np.float32)
    for c in range(NCORES):
        out[:, c * IS:(c + 1) * IS, :] = results[c]["out"]
    return out


def kernel(**inputs) -> np.ndarray:
    from concourse.bass_utils import run_bass_kernel_spmd

    nc = _get_program()
    in_maps = make_in_maps(inputs)
    res = run_bass_kernel_spmd(nc, in_maps, core_ids=list(range(NCORES)))
    return gather_out(res.results)



# revision 53
# speedup vs baseline: 1.1336x; 1.1336x over previous
"""EquivariantAttention Trainium2 kernel.

Reference computation (B=4, S=512, D=512, H=8, DH=64):
    qkv = x @ W_qkv                      -> q, k, v  (b, s, h, dh)
    geo_w = geometric_features @ W_geo   -> (b, h, i, j)
    pos_w = positional_encodings @ W_pos -> (h, i, j)
    scores = q k^T / sqrt(dh) + geo_w + pos_w
    attn   = softmax_j(scores)            (mask is all-ones -> no-op)
    out    = (attn @ v) @ W_out

Sharding: the 512 MB positional_encodings table dominates HBM traffic, so the
query dim i is sharded across the 8 cores (64 rows each).  Every core computes
its own i-slice of the output; the host concats.

Design notes (per core):
- Scores live TRANSPOSED — j on partitions, (h, i) in the free dim — because
  pos_w can only leave the tensor engine as out[M=j_tile, N=h] with d on the
  contraction partitions.  The host stages positional_encodings pre-transposed
  to (d, i, j) and pre-cast to fp8 e3m4 (4 mantissa bits keep the pos_w
  quantization error ~0.9%), quartering the dominant HBM stream vs f32; the
  fp8 stationary x bf16 W_pos mix is exact on the PE.
- The k projection over all 2048 tokens never happens: scores are computed as
  x^T (W_k q) — Z = W_k q is an 8-head batch of tiny matmuls, and the j-side
  contraction reuses the xT tiles already resident for the q/v projections.
- geo_w is combined with pos_w into per-(b, j-block) `posgeo` bias tiles
  (wide batched DVE/Pool ops), so the softmax tail is just add + exp per
  (b, jb) — no separate geo multiply.  pos_w accumulates in two i-halves so
  half the bias work runs under the DMA stream.
- v tiles carry a ones column per head: attn @ v and the softmax denominator
  come out of one matmul pass.  Softmax over j (= partitions) skips
  max-subtraction (scores are O(1)).  The out-projection runs per batch,
  overlapping the next batch's attention.
- The whole body sits in a hardware For_i loop (NREPS iterations per NEFF
  execution, all-engine barrier + sem reset between iterations).  Each
  iteration is a complete independent kernel run — the loop only amortizes
  the multi-ms axon per-call dispatch overhead so test.py can measure
  per-iteration HW latency precisely.
"""

import numpy as np

B, S, D, H = 4, 512, 512, 8
DH = D // H            # 64
NCORES = 8
IS = S // NCORES       # 64  i-rows per core
T = B * S              # 2048 tokens
TI = B * IS            # 256 slice tokens
IGRP = 8               # i-rows per P-load DMA group
# In-NEFF repetition count: the body below is emitted once and wrapped in a
# hardware For_i loop, so one NEFF execution runs the full kernel NREPS times
# back-to-back (all-engine barrier + sem reset between iterations — no
# overlap).  Each iteration is a complete, self-contained kernel run; the
# repetition only amortizes the multi-ms per-call dispatch overhead of the
# axon-tunnelled PJRT path, letting test.py measure per-iteration HW latency.
NREPS = 1024

_CACHE = {}


def _build_program(nreps: int = NREPS):
    from contextlib import ExitStack

    import concourse.bacc as bacc
    import concourse.mybir as mybir
    import concourse.tile as tile
    from concourse.masks import make_identity

    f32 = mybir.dt.float32
    bf16 = mybir.dt.bfloat16
    AF = mybir.ActivationFunctionType
    ALU = mybir.AluOpType

    nc = bacc.Bacc(
        "TRN2",
        target_bir_lowering=False,
        debug=False,
        enable_asserts=False,
        num_devices=NCORES,
    )

    x_d = nc.dram_tensor("x", [D, T], bf16, kind="ExternalInput").ap()
    xs_d = nc.dram_tensor("x_slice", [D, TI], bf16, kind="ExternalInput").ap()
    # W_k^T, head-major: row dh, col h*512 + d  (the k-block of W_qkv,
    # host-transposed).  Scores are computed as x^T (W_k q) instead of
    # (W_k^T x)·q — the k projection over all 2048 tokens never happens.
    wkT_d = nc.dram_tensor("w_kT", [DH, H * D], bf16, kind="ExternalInput").ap()
    f8 = mybir.dt.float8e3
    # positional_encodings arrive host-pre-transposed to (d, i, j) so the
    # contraction dim d lands on SBUF partitions straight out of the DMA;
    # fp8 e3m4 halves the dominant HBM stream (4 mantissa bits keep the
    # pos_w quantization error ~0.9%; pos values ~N(0,1) fit the e3m4
    # range, and W_pos is pre-scaled by POS_SCALE to reach normals)
    p_d = nc.dram_tensor("pos_enc", [D, IS, S], f8, kind="ExternalInput").ap()
    g_d = nc.dram_tensor("geo", [B, 2, IS, S], bf16, kind="ExternalInput").ap()
    # q and v blocks only (k is folded into wkT): cols 0:512 = W_q, 512:1024 = W_v
    wqkv_d = nc.dram_tensor("w_qkv", [D, 2 * D], bf16, kind="ExternalInput").ap()
    wpos_d = nc.dram_tensor("w_pos", [D, H], bf16, kind="ExternalInput").ap()
    wgeo_d = nc.dram_tensor("w_geo", [2, H], f32, kind="ExternalInput").ap()
    wout_d = nc.dram_tensor("w_out", [D, D], bf16, kind="ExternalInput").ap()
    out_d = nc.dram_tensor("out", [B, IS, D], f32, kind="ExternalOutput").ap()

    with tile.TileContext(nc) as tc, ExitStack() as ctx:
        if nreps > 1:
            ctx.enter_context(tc.For_i(0, nreps))
        # ---------------- Phase 0: constants ----------------
        cp = ctx.enter_context(tc.tile_pool(name="consts", bufs=1))

        ident = cp.tile([128, 128], bf16, name="ident", tag="ident")
        make_identity(nc, ident)

        ones_col = cp.tile([128, 1], bf16, name="ones_col", tag="ones_col")
        nc.gpsimd.memset(ones_col, 1.0)

        wqkv_sb = []
        for dt_ in range(4):
            t_ = cp.tile([128, 2 * D], bf16, name=f"wqkv_{dt_}", tag=f"wqkv{dt_}")
            nc.scalar.dma_start(out=t_, in_=wqkv_d[dt_ * 128:(dt_ + 1) * 128, :])
            wqkv_sb.append(t_)

        wkT_sb = cp.tile([DH, H * D], bf16, name="wkT_sb", tag="wkT")
        nc.scalar.dma_start(out=wkT_sb, in_=wkT_d)

        # W_pos as (128, 4*8): [:, db*8:(db+1)*8] = rows db*128..db*128+127
        # (bf16 moving operand vs fp8 stationary table: PE allows the mix,
        # and keeping the weight exact halves the pos_w quantization error)
        wpos_sb = cp.tile([128, 32], bf16, name="wpos_sb", tag="wpos")
        nc.scalar.dma_start(
            out=wpos_sb.rearrange("p (a h) -> p a h", a=4),
            in_=wpos_d.rearrange("(a p) h -> p a h", p=128),
        )

        # W_geo broadcast to all partitions: (128, 16) f32, col c*8+h
        wgeo_flat = cp.tile([1, 16], f32, name="wgeo_flat", tag="wgf")
        nc.scalar.dma_start(
            out=wgeo_flat, in_=wgeo_d.rearrange("c h -> (c h)")[None, :])
        wgeo_fbf = cp.tile([1, 16], bf16, name="wgeo_fbf", tag="wgfb")
        nc.vector.tensor_copy(wgeo_fbf, wgeo_flat)
        wgeo_bc = cp.tile([128, 16], f32, name="wgeo_bc", tag="wgbc")
        ones_r128 = cp.tile([1, 128], bf16, name="ones_r128", tag="ones_r128")
        nc.gpsimd.memset(ones_r128, 1.0)
        with tc.tile_pool(name="bc_ps2", bufs=1, space="PSUM") as bc_pool:
            bc_ps = bc_pool.tile([128, 16], f32, name="bc_ps2", tag="bcps2")
            nc.tensor.matmul(bc_ps, ones_r128, wgeo_fbf, start=True, stop=True)
            nc.vector.tensor_copy(wgeo_bc, bc_ps)

        wout_sb = []
        for db in range(4):
            t_ = cp.tile([128, D], bf16, name=f"wout_{db}", tag=f"wout{db}")
            nc.scalar.dma_start(out=t_, in_=wout_d[db * 128:(db + 1) * 128, :])
            wout_sb.append(t_)

        # ---------------- Phase 1: xT, xsT straight from host ------------
        # x arrives host-transposed (d, tokens) bf16: plain DMAs, no PE work
        xT_sb = [cp.tile([128, T], bf16, name=f"xT_{db}", tag=f"xT{db}")
                 for db in range(4)]
        xsT_sb = [cp.tile([128, TI], bf16, name=f"xsT_{db}", tag=f"xsT{db}")
                  for db in range(4)]
        for db in range(4):
            nc.scalar.dma_start(out=xT_sb[db], in_=x_d[db * 128:(db + 1) * 128, :])
            nc.scalar.dma_start(out=xsT_sb[db], in_=xs_d[db * 128:(db + 1) * 128, :])

        # ---------------- Phase 2 decls (work emitted inside phase 4) ----
        # v tiles carry 65 cols per head: 64 v values + a ones column, so the
        # attn matmul computes the softmax denominator in the same pass
        v_sb = [cp.tile([128, H * 65], bf16, name=f"v_{tt}", tag=f"v{tt}")
                for tt in range(T // 128)]
        qT_sb = [cp.tile([DH, TI], bf16, name=f"qT_{h}", tag=f"qT{h}")
                 for h in range(H)]
        # Z[db] : (128 d-chunk, (b,h,i) 2048) bf16 = W_k q (scores stage 1)
        z_sb = [cp.tile([128, B * 512], bf16, name=f"z_{db}", tag=f"z{db}")
                for db in range(4)]
        # qk_sb[b][jb] : (128 j, (h,i) 512) bf16 raw q·k/8 scores
        qk_sb = [[cp.tile([128, 512], bf16, name=f"qk{b}_{jb}",
                          tag=f"qk{b}{jb}") for jb in range(4)]
                 for b in range(B)]

        def _emit_v(proj_pool, tt):
            ps = proj_pool.tile([128, 512], f32, name="ps_v", tag="pj")
            for dt_ in range(4):
                nc.tensor.matmul(
                    ps,
                    xT_sb[dt_][:, tt * 128:(tt + 1) * 128],
                    wqkv_sb[dt_][:, 512:1024],
                    start=(dt_ == 0), stop=(dt_ == 3),
                )
            vv = v_sb[tt].rearrange("p (h c) -> p h c", c=65)
            pv = ps.rearrange("p (h d) -> p h d", d=DH)
            if tt % 2 == 0:
                nc.vector.tensor_copy(vv[:, :, 0:DH], pv)
            else:
                nc.scalar.copy(vv[:, :, 0:DH], pv)
            nc.gpsimd.memset(vv[:, :, DH:65], 1.0)

        def _emit_qT(proj_pool, h):
            ps = proj_pool.tile([DH, TI], f32, name="ps_q", tag="pj")
            for dt_ in range(4):
                nc.tensor.matmul(
                    ps,
                    wqkv_sb[dt_][:, h * DH:(h + 1) * DH],
                    xsT_sb[dt_],
                    start=(dt_ == 0), stop=(dt_ == 3),
                )
            nc.scalar.mul(qT_sb[h], ps, 0.125)   # fold 1/sqrt(DH)

        def _emit_z(proj_pool, h, db):
            ps = proj_pool.tile([128, TI], f32, name="ps_z", tag="pj")
            nc.tensor.matmul(
                ps,
                wkT_sb[:, h * D + db * 128:h * D + (db + 1) * 128],
                qT_sb[h],
                start=True, stop=True,
            )
            zv = z_sb[db].rearrange(
                "p (b h i) -> p b h i", h=H, i=IS)[:, :, h, :]
            pv = ps.rearrange("p (b i) -> p b i", i=IS)
            if (h + db) % 2 == 0:
                nc.vector.tensor_copy(zv, pv)
            else:
                nc.scalar.copy(zv, pv)

        def _emit_s(proj_pool, b, jb):
            ps = proj_pool.tile([128, 512], f32, name="ps_s", tag="pj")
            for db in range(4):
                nc.tensor.matmul(
                    ps,
                    xT_sb[db][:, b * S + jb * 128:b * S + (jb + 1) * 128],
                    z_sb[db][:, b * 512:(b + 1) * 512],
                    start=(db == 0), stop=(db == 3),
                )
            if (b + jb) % 2 == 0:
                nc.vector.tensor_copy(qk_sb[b][jb], ps)
            else:
                nc.scalar.copy(qk_sb[b][jb], ps)

        # geo: the two channel planes are PE-transposed into two WIDE tiles
        # (all 16 (b,jb) units side by side), the per-head combine then runs
        # as 16 full-width DVE ops instead of 256 64-col ones, and the result
        # is pre-ADDED to pos_w (posgeo tiles) so the softmax tail needs no
        # separate geo multiply and no geo exp.
        gT_all = [cp.tile([128, 1024], bf16, name=f"gT_all{c}", tag=f"gA{c}")
                  for c in range(2)]
        t2_all = cp.tile([128, 1024], f32, name="t2_all", tag="t2A")
        gtmp_all = cp.tile([128, 8192], bf16, name="gtmp_all", tag="gtA")
        posgeo = [[cp.tile([128, 512], bf16, name=f"pg{b}_{jb}",
                           tag=f"pg{b}{jb}") for jb in range(4)]
                  for b in range(B)]
        g_nat = []
        for b in range(B):
            gn2 = []
            for c in range(2):
                gn = cp.tile([IS, S], bf16, name=f"g_nat{b}_{c}", tag=f"gn{b}{c}")
                nc.scalar.dma_start(out=gn, in_=g_d[b, c])
                gn2.append(gn)
            g_nat.append(gn2)

        def _emit_geoT(proj_pool, b, jb):
            u = b * 4 + jb
            for c in range(2):
                # PE transpose (not xbar DMA-transpose: mixing DmaTranspose
                # with the P-load DMA stream forces xbar-mode serialization);
                # psum comes from the shared proj pool's slots
                gt_ps = proj_pool.tile([128, IS], bf16,
                                       name=f"gt_ps{c}", tag="pj")
                nc.tensor.transpose(
                    gt_ps, g_nat[b][c][:, jb * 128:(jb + 1) * 128],
                    ident[0:IS, 0:IS])
                dst = gT_all[c][:, u * IS:(u + 1) * IS]
                if c == 0:
                    nc.vector.tensor_copy(dst, gt_ps)
                else:
                    nc.scalar.copy(dst, gt_ps)

        def _emit_gtmp(h):
            gv = gtmp_all.rearrange("p (u h i) -> p u h i", h=H, i=IS)
            g0 = gT_all[0].rearrange("p (u i) -> p u i", i=IS)
            g1 = gT_all[1].rearrange("p (u i) -> p u i", i=IS)
            t2 = t2_all.rearrange("p (u i) -> p u i", i=IS)
            nc.vector.tensor_scalar(
                t2, g1, wgeo_bc[:, 8 + h:9 + h], None, op0=ALU.mult)
            nc.vector.scalar_tensor_tensor(
                gv[:, :, h, :], g0, wgeo_bc[:, h:h + 1], t2,
                op0=ALU.mult, op1=ALU.add)

        pos_sb = [cp.tile([128, 512], bf16, name=f"pos_{jb}", tag=f"pos{jb}")
                  for jb in range(4)]

        def _emit_poscopy(half, jb):
            # single strided (i,h) -> (h,i) re-layout pass per (jb, half);
            # everything downstream then reads contiguous blocks
            hi0 = half * (IS // 2)
            hi1 = hi0 + IS // 2
            pp = pos_ps[jb].rearrange("p (i h) -> p h i", h=H)[:, :, hi0:hi1]
            og = pos_sb[jb].rearrange("p (h i) -> p h i", i=IS)[:, :, hi0:hi1]
            nc.vector.tensor_copy(og, pp)

        def _emit_posgeo(half, b, jb):
            # b-dependent geo bias add; mid-stream half rides the idle Pool
            # engine, the latency-critical tail half alternates Pool/DVE
            hi0 = half * (IS // 2)
            hi1 = hi0 + IS // 2
            u = b * 4 + jb
            pp = pos_sb[jb].rearrange("p (h i) -> p h i", i=IS)[:, :, hi0:hi1]
            gg = gtmp_all.rearrange(
                "p (u h i) -> p u h i", h=H, i=IS)[:, u, :, hi0:hi1]
            og = posgeo[b][jb].rearrange(
                "p (h i) -> p h i", i=IS)[:, :, hi0:hi1]
            eng = nc.gpsimd if (half == 0 or (b + jb) % 2 == 0) else nc.vector
            eng.tensor_add(og, pp, gg)

        proj_items = (
            [lambda pp, tt=tt: _emit_v(pp, tt) for tt in range(T // 128)]
            + [it
               for h in range(H)
               for it in ([lambda pp, h=h: _emit_qT(pp, h)]
                          + [lambda pp, h=h, db=db: _emit_z(pp, h, db)
                             for db in range(4)])]
            + [lambda pp, b=b, jb=jb: _emit_s(pp, b, jb)
               for b in range(B) for jb in range(4)]
        )
        geoT_items = [(b, jb) for b in range(B) for jb in range(4)]

        # ---------------- Phase 4: pos_w + interleaved projections -------
        # The PE stream is in-order: interleaving projection matmuls between
        # each i-group's (DMA-gated) pos matmuls lets the PE fill DMA-wait
        # gaps with useful work instead of stalling.  pos_ps accumulates in
        # two i-halves so the posgeo adds for i<32 run mid-stream.
        with tc.tile_pool(name="pos_ps", bufs=1, space="PSUM") as pos_pool:
            pos_ps = [pos_pool.tile([128, 512], f32, name=f"pos_ps{jb}",
                                    tag=f"pps{jb}") for jb in range(4)]
            with tc.tile_pool(name="p_t", bufs=12) as pt_pool, \
                 tc.tile_pool(name="proj_ps", bufs=4, space="PSUM") as proj_pool:
                n_groups = IS // IGRP
                per_grp = (len(proj_items) + n_groups - 1) // n_groups
                HALF = IS // 2
                for gi, i0 in enumerate(range(0, IS, IGRP)):
                    ptg = []
                    for db in range(4):
                        pt = pt_pool.tile([128, IGRP * 512], f8,
                                          name="ptg", tag="ptg")
                        nc.sync.dma_start(
                            out=pt.rearrange("p (a j) -> p a j", a=IGRP),
                            in_=p_d[db * 128:(db + 1) * 128, i0:i0 + IGRP, :],
                        )
                        ptg.append(pt)
                    for it in proj_items[gi * per_grp:(gi + 1) * per_grp]:
                        it(proj_pool)
                    if gi < 3:
                        for b, jb in geoT_items[gi * 6:(gi + 1) * 6]:
                            _emit_geoT(proj_pool, b, jb)
                    elif gi == 3:
                        for h in range(H):
                            _emit_gtmp(h)
                    elif gi == 4:
                        for jb in range(4):
                            _emit_poscopy(0, jb)
                    elif gi == 5:
                        for b in range(B):
                            for jb in range(4):
                                _emit_posgeo(0, b, jb)
                    for a in range(IGRP):
                        i = i0 + a
                        for jb in range(4):
                            for db in range(4):
                                nc.tensor.matmul(
                                    pos_ps[jb][:, i * 8:(i + 1) * 8],
                                    ptg[db][:, a * 512 + jb * 128:
                                            a * 512 + (jb + 1) * 128],
                                    wpos_sb[:, db * 8:(db + 1) * 8],
                                    # one psum group per (bank, i-half): the
                                    # half's first i starts, its last i
                                    # stops; each i's first db write lands on
                                    # pending-zero bytes and overwrites,
                                    # later dbs accumulate.
                                    start=(i % HALF == 0 and db == 0),
                                    stop=(i % HALF == HALF - 1 and db == 3),
                                )
            for jb in range(4):
                _emit_poscopy(1, jb)
            for b in range(B):
                for jb in range(4):
                    _emit_posgeo(1, b, jb)

        # ---------------- Phase 5: scores, softmax, attn @ v -------------
        O_sb = [cp.tile([IS, D], bf16, name=f"O_{b}", tag=f"O{b}") for b in range(B)]
        with tc.tile_pool(name="o_ps", bufs=1, space="PSUM") as o_pool, \
             tc.tile_pool(name="ot_ps", bufs=2, space="PSUM") as ot_pool, \
             tc.tile_pool(name="f_ps", bufs=2, space="PSUM") as f_pool, \
             tc.tile_pool(name="att_sb", bufs=3) as att_pool:
            for b in range(B):
                o_ps2 = [o_pool.tile([IS, 4 * 65], f32, name=f"o_ps{g}",
                                     tag=f"ops{g}", bufs=2) for g in range(2)]
                for jb in range(4):
                    t1 = att_pool.tile([128, 512], f32, name="t1", tag="t1")
                    nc.vector.tensor_add(t1, qk_sb[b][jb], posgeo[b][jb])
                    e1 = att_pool.tile([128, 512], bf16, name="e1", tag="e1")
                    nc.scalar.activation(e1, t1, AF.Exp)
                    tt = b * 4 + jb
                    for h in range(H):
                        g, hg = divmod(h, 4)
                        nc.tensor.matmul(
                            o_ps2[g][:, hg * 65:(hg + 1) * 65],
                            e1[:, h * IS:(h + 1) * IS],
                            v_sb[tt][:, h * 65:(h + 1) * 65],
                            start=(jb == 0 and hg == 0),
                            stop=(jb == 3 and hg == 3),
                        )
                recip = att_pool.tile([IS, H], f32, name="recip", tag="recip")
                for g in range(2):
                    nc.vector.reciprocal(
                        recip[:, g * 4:(g + 1) * 4],
                        o_ps2[g].rearrange("p (h c) -> p h c", c=65)[:, :, DH])
                for h in range(H):
                    g, hg = divmod(h, 4)
                    src = o_ps2[g][:, hg * 65:hg * 65 + DH]
                    if h % 2 == 0:
                        nc.vector.tensor_scalar(
                            O_sb[b][:, h * DH:(h + 1) * DH], src,
                            recip[:, h:h + 1], None, op0=ALU.mult)
                    else:
                        nc.scalar.activation(
                            O_sb[b][:, h * DH:(h + 1) * DH], src,
                            AF.Copy, scale=recip[:, h:h + 1])
                # out-proj for this b right away: overlaps later batches'
                # attention instead of forming a serial phase at the end
                otb = []
                for db in range(4):
                    ot_ps_t = ot_pool.tile([128, IS], bf16, name="otps",
                                           tag="otps")
                    nc.tensor.transpose(
                        ot_ps_t, O_sb[b][:, db * 128:(db + 1) * 128],
                        ident[0:IS, 0:IS])
                    ot = att_pool.tile([128, IS], bf16, name=f"ot{db}",
                                       tag=f"ot{db}", bufs=2)
                    if db % 2 == 0:
                        nc.vector.tensor_copy(ot, ot_ps_t)
                    else:
                        nc.scalar.copy(ot, ot_ps_t)
                    otb.append(ot)
                f_ps = f_pool.tile([IS, D], f32, name="f_ps", tag="fps")
                for db in range(4):
                    nc.tensor.matmul(
                        f_ps, otb[db], wout_sb[db],
                        start=(db == 0), stop=(db == 3),
                    )
                fout = att_pool.tile([IS, D], f32, name="fout", tag="fout",
                                     bufs=2)
                nc.vector.tensor_copy(fout, f_ps)
                nc.sync.dma_start(out=out_d[b], in_=fout)

    nc.compile()
    return nc


def _get_program(nreps: int = NREPS):
    key = f"nc{nreps}"
    if key not in _CACHE:
        _CACHE[key] = _build_program(nreps)
    return _CACHE[key]


def make_in_maps(inputs):
    import ml_dtypes
    bf = ml_dtypes.bfloat16
    f8 = ml_dtypes.float8_e3m4
    x = np.asarray(inputs["x"], np.float32)                       # (B, S, D)
    geo = np.asarray(inputs["geometric_features"], np.float32)    # (B, S, S, 2)
    pos = np.asarray(inputs["positional_encodings"], np.float32)  # (S, S, D)
    wqkv = np.asarray(inputs["W_qkv"], np.float32)
    wout = np.asarray(inputs["W_out"], np.float32)
    wgeo = np.asarray(inputs["W_geo"], np.float32)
    wpos = np.asarray(inputs["W_pos"], np.float32)
    mask = np.asarray(inputs["mask"])

    assert mask.all(), "kernel assumes an all-true mask"
    for k in ("b_qkv", "b_out", "b_geo", "b_pos"):
        assert not np.asarray(inputs[k], np.float32).any(), \
            "kernel assumes zero biases (reference setup_inputs uses zeros)"

    # big inputs staged as bf16 on the host: halves device HBM traffic and
    # makes every load a plain HWDGE DMA (matmuls consume bf16 anyway, so
    # numerics match the previous cast-during-DMA scheme)
    x_flat = np.ascontiguousarray(x.reshape(T, D).T.astype(bf))
    # q and v blocks side by side; the k block ships transposed (head-major)
    wqkv_b = np.ascontiguousarray(
        np.concatenate([wqkv[:, :D], wqkv[:, 2 * D:]], axis=1).astype(bf))
    wkT_b = np.ascontiguousarray(
        wqkv[:, D:2 * D].reshape(D, H, DH).transpose(2, 1, 0)
        .reshape(DH, H * D).astype(bf))
    wpos_b = np.ascontiguousarray(wpos.astype(bf))
    wout_b = np.ascontiguousarray(wout.astype(bf))
    in_maps = []
    for c in range(NCORES):
        lo = c * IS
        in_maps.append({
            "x": x_flat,
            "x_slice": np.ascontiguousarray(
                x[:, lo:lo + IS].reshape(TI, D).T.astype(bf)),
            "pos_enc": np.ascontiguousarray(
                pos[lo:lo + IS].transpose(2, 0, 1).astype(f8)),
            "geo": np.ascontiguousarray(
                geo[:, lo:lo + IS].transpose(0, 3, 1, 2).astype(bf)),
            "w_qkv": wqkv_b,
            "w_kT": wkT_b,
            "w_pos": wpos_b,
            "w_geo": wgeo,
            "w_out": wout_b,
        })
    return in_maps


def gather_out(results):
    out = np.empty((B, S, D), np.float32)
    for c in range(NCORES):
        out[:, c * IS:(c + 1) * IS, :] = results[c]["out"]
    return out


def kernel(**inputs) -> np.ndarray:
    from concourse.bass_utils import run_bass_kernel_spmd

    nc = _get_program()
    in_maps = make_in_maps(inputs)
    res = run_bass_kernel_spmd(nc, in_maps, core_ids=list(range(NCORES)))
    return gather_out(res.results)

